# revision 8
# baseline (speedup 1.0000x reference)
"""Trainium2 Bass kernel for nn_LinearCritic (SimCLR-style loss scores).

Pipeline (reference): for each of z1,z2 [4096,2048]:
  X = z @ W1.T ; X = BN(X)*gamma+beta ; Y = relu(X) ; U = Y @ W2.T ; P = BN(U)
then cosine-similarity blocks between normalized projections form a
[8192, 8192] score matrix (diag of s00/s11 = -inf), targets = arange.

Sharding: batch rows split across 8 cores (512 rows of z1 + 512 of z2 each).
BatchNorm batch statistics are global -> tiny AllReduces; the 128-d
normalized projections are AllGathered; each core then computes its
1024-row block of the output (memory-bound: 32 MB of the 256 MB output).

Schedule: z1's whole chain (GEMM1 -> BN-stats AllReduce -> relu -> GEMM2 ->
BN2 AllReduce -> normalize -> AllGather) is pipelined under z2's GEMM1 so
the collective latencies overlap with PE work. Matmuls run in float32r
(tf32-like, 11-bit mantissa); everything else is fp32.
"""
import sys

sys.path.insert(0, "/opt/trn_rl_repo")

import numpy as np

import concourse.bass as bass
import concourse.bacc as bacc
import concourse.mybir as mybir
import concourse.tile as tile
from concourse.bass_utils import run_bass_kernel_spmd

NCORES = 8
N = 4096          # batch rows per z tensor
D = 2048          # hidden dim
P = 128           # projection dim
R = N // NCORES   # 512 rows per core per z
KT = D // 128     # 16 k/j tiles
TEMP_SCALE = 2.0  # 1/TEMPERATURE
BN_EPS = 1e-5

f32 = mybir.dt.float32
f32r = mybir.dt.float32r

_CACHE = {}


def _build(dbg=False):
    nc = bacc.Bacc("TRN2", target_bir_lowering=False, debug=False,
                   num_devices=NCORES)

    zT = nc.dram_tensor("zT", [D, 2 * R], f32r, kind="ExternalInput")
    W1c = nc.dram_tensor("W1c", [KT, D, 128], f32r, kind="ExternalInput")
    W2c = nc.dram_tensor("W2c", [D, 128], f32r, kind="ExternalInput")
    gb = nc.dram_tensor("gb", [128, 2 * KT], f32, kind="ExternalInput")

    out_top = nc.dram_tensor("out_top", [R, 2 * N], f32, kind="ExternalOutput")
    out_bot = nc.dram_tensor("out_bot", [R, 2 * N], f32, kind="ExternalOutput")

    ar1_in = [nc.dram_tensor(f"ar1_in{z}", [128, 2 * KT], f32, kind="Internal")
              for z in range(2)]
    ar1_out = [nc.dram_tensor(f"ar1_out{z}", [128, 2 * KT], f32,
                              kind="Internal", addr_space="Shared")
               for z in range(2)]
    ar2_in = [nc.dram_tensor(f"ar2_in{z}", [128, 2], f32, kind="Internal")
              for z in range(2)]
    ar2_out = [nc.dram_tensor(f"ar2_out{z}", [128, 2], f32,
                              kind="Internal", addr_space="Shared")
               for z in range(2)]
    ag_in = [nc.dram_tensor(f"ag_in{z}", [128, R], f32, kind="Internal")
             for z in range(2)]
    ag_out = [nc.dram_tensor(f"ag_out{z}", [NCORES * 128, R], f32,
                             kind="Internal", addr_space="Shared")
              for z in range(2)]

    dbg_t = {}
    if dbg:
        dbg_t["qloc"] = nc.dram_tensor("dbg_qloc", [128, 2 * R], f32,
                                       kind="ExternalOutput")
        dbg_t["qfull"] = nc.dram_tensor("dbg_qfull", [128, 2 * N], f32,
                                        kind="ExternalOutput")

    rg = [list(range(NCORES))]
    AF = mybir.ActivationFunctionType
    ALU = mybir.AluOpType

    with tile.TileContext(nc) as tc:
        with tc.tile_pool(name="small", bufs=1) as small, \
             tc.tile_pool(name="persist", bufs=1) as persist, \
             tc.tile_pool(name="uu", bufs=2) as up, \
             tc.tile_pool(name="ph", bufs=2) as php, \
             tc.tile_pool(name="sq", bufs=2) as sqp, \
             tc.tile_pool(name="nrm", bufs=2) as nrmp, \
             tc.tile_pool(name="psn", bufs=1, space="PSUM") as ppn, \
             tc.tile_pool(name="psb", bufs=1, space="PSUM") as ppb:

            # persistent small tiles
            stats1 = small.tile([128, 4 * KT], f32, tag="stats1")
            stats1_g = small.tile([128, 4 * KT], f32, tag="stats1g")
            gb_sb = small.tile([128, 2 * KT], f32, tag="gb")
            scale1 = small.tile([128, 2 * KT], f32, tag="scale1")
            shift1 = small.tile([128, 2 * KT], f32, tag="shift1")
            stats2 = small.tile([128, 4], f32, tag="stats2")
            stats2_g = small.tile([128, 4], f32, tag="stats2g")
            istd2 = small.tile([128, 2], f32, tag="istd2")
            shift2 = small.tile([128, 2], f32, tag="shift2")
            ones_k = small.tile([128, 1], f32r, tag="ones_k")
            ones_m = small.tile([1, 128], f32r, tag="ones_m")
            qloc = persist.tile([128, 2 * R], f32r, tag="qloc")
            eps_sb = small.tile([128, 1], f32, tag="eps")
            ones_k32 = small.tile([128, 1], f32, tag="ones_k32")
            ones_m32 = small.tile([1, 128], f32, tag="ones_m32")

            nc.sync.dma_start(gb_sb[:], gb.ap()[:])
            nc.vector.memset(ones_k32[:], 1.0)
            nc.vector.memset(ones_m32[:], 1.0)
            nc.vector.tensor_copy(ones_k[:], ones_k32[:])
            nc.vector.tensor_copy(ones_m[:], ones_m32[:])
            nc.vector.memset(eps_sb[:], BN_EPS)

            with tc.tile_pool(name="w1", bufs=2) as w1p, \
                 tc.tile_pool(name="ztp", bufs=1) as ztp, \
                 tc.tile_pool(name="xx", bufs=16) as xp, \
                 tc.tile_pool(name="yy", bufs=16) as yp, \
                 tc.tile_pool(name="w2", bufs=1) as w2p, \
                 tc.tile_pool(name="scr", bufs=2) as scrp, \
                 tc.tile_pool(name="ps13", bufs=3, space="PSUM") as pp13:

                # first weight block before the big z load, so PE can start
                # as soon as zT lands
                w1_first = w1p.tile([128, KT, 128], f32r, tag="w1")
                nc.sync.dma_start(
                    w1_first[:], W1c.ap()[0].rearrange("(k p) c -> p k c", p=128))
                ztile = ztp.tile([128, KT, 2 * R], f32r, tag="zt")
                nc.sync.dma_start(
                    ztile[:], zT.ap().rearrange("(k p) n -> p k n", p=128))
                w2sb = w2p.tile([128, KT, 128], f32r, tag="w2")
                nc.sync.dma_start(
                    w2sb[:], W2c.ap().rearrange("(k p) c -> p k c", p=128))

                xs = {}
                ys = {}
                us = {}

                for zi in range(2):
                    zsl = slice(R * zi, R * (zi + 1))
                    # ---- GEMM1 for this z + BN1 partial sums ----
                    for j in range(KT):
                        if zi == 0 and j == 0:
                            w1j = w1_first
                        else:
                            w1j = w1p.tile([128, KT, 128], f32r, tag="w1")
                            nc.sync.dma_start(
                                w1j[:],
                                W1c.ap()[j].rearrange("(k p) c -> p k c", p=128))
                        xj = xp.tile([128, R], f32, tag="xx")
                        xs[(zi, j)] = xj
                        ps = pp13.tile([128, R], f32, tag="ps13")
                        for k in range(KT):
                            nc.tensor.matmul(
                                ps[:], w1j[:, k, :], ztile[:, k, zsl],
                                start=(k == 0), stop=(k == KT - 1))
                        nc.vector.tensor_scalar(
                            xj[:], ps[:], 0.0, 0.0, ALU.add, ALU.add,
                            accum_out=stats1[:, 2 * KT * zi + j:
                                             2 * KT * zi + j + 1])
                        scr = scrp.tile([128, R], f32, tag="scr")
                        nc.scalar.activation(
                            scr[:], ps[:], AF.Square,
                            accum_out=stats1[:, 2 * KT * zi + KT + j:
                                             2 * KT * zi + KT + j + 1])

                    # ---- AllReduce BN1 stats for this z ----
                    nc.sync.dma_start(ar1_in[zi].ap()[:],
                                      stats1[:, 2 * KT * zi:2 * KT * (zi + 1)])
                    nc.gpsimd.collective_compute(
                        "AllReduce", ALU.add, replica_groups=rg,
                        ins=[ar1_in[zi].ap()], outs=[ar1_out[zi].ap()])
                    nc.sync.dma_start(
                        stats1_g[:, 2 * KT * zi:2 * KT * (zi + 1)],
                        ar1_out[zi].ap()[:])

                    # ---- BN1 coefficients for this z ----
                    csl = slice(KT * zi, KT * (zi + 1))
                    mean1 = small.tile([128, KT], f32, tag=f"mean1_{zi}")
                    var1 = small.tile([128, KT], f32, tag=f"var1_{zi}")
                    tmp1 = small.tile([128, KT], f32, tag=f"tmp1_{zi}")
                    nc.vector.tensor_scalar_mul(
                        mean1[:], stats1_g[:, 2 * KT * zi:2 * KT * zi + KT],
                        1.0 / N)
                    nc.vector.tensor_scalar_mul(
                        var1[:], stats1_g[:, 2 * KT * zi + KT:2 * KT * (zi + 1)],
                        1.0 / N)
                    nc.vector.tensor_mul(tmp1[:], mean1[:], mean1[:])
                    nc.vector.tensor_sub(var1[:], var1[:], tmp1[:])
                    nc.scalar.activation(tmp1[:], var1[:], AF.Sqrt, bias=eps_sb[:])
                    nc.vector.reciprocal(var1[:], tmp1[:])  # var1 := istd
                    nc.vector.tensor_mul(scale1[:, csl], var1[:], gb_sb[:, 0:KT])
                    nc.vector.tensor_mul(tmp1[:], mean1[:], scale1[:, csl])
                    nc.vector.tensor_sub(shift1[:, csl], gb_sb[:, KT:2 * KT],
                                         tmp1[:])

                    # ---- relu + GEMM2 for this z ----
                    for j in range(KT):
                        yj = yp.tile([128, R], f32r, tag="yy")
                        ys[(zi, j)] = yj
                        nc.scalar.activation(
                            yj[:], xs[(zi, j)][:], AF.Relu,
                            scale=scale1[:, KT * zi + j:KT * zi + j + 1],
                            bias=shift1[:, KT * zi + j:KT * zi + j + 1])
                    psu = pp13.tile([128, R], f32, tag="ps13")
                    for j in range(KT):
                        nc.tensor.matmul(
                            psu[:], w2sb[:, j, :], ys[(zi, j)][:],
                            start=(j == 0), stop=(j == KT - 1))
                    un = up.tile([128, R], f32, tag="uu")
                    us[zi] = un
                    nc.vector.tensor_scalar(
                        un[:], psu[:], 0.0, 0.0, ALU.add, ALU.add,
                        accum_out=stats2[:, 2 * zi:2 * zi + 1])
                    scr = scrp.tile([128, R], f32, tag="scr")
                    nc.scalar.activation(
                        scr[:], psu[:], AF.Square,
                        accum_out=stats2[:, 2 * zi + 1:2 * zi + 2])

                    # ---- AllReduce BN2 stats for this z ----
                    nc.sync.dma_start(ar2_in[zi].ap()[:],
                                      stats2[:, 2 * zi:2 * (zi + 1)])
                    nc.gpsimd.collective_compute(
                        "AllReduce", ALU.add, replica_groups=rg,
                        ins=[ar2_in[zi].ap()], outs=[ar2_out[zi].ap()])
                    nc.sync.dma_start(stats2_g[:, 2 * zi:2 * (zi + 1)],
                                      ar2_out[zi].ap()[:])

                    # ---- BN2 coefficients (affine=False) ----
                    mean2 = small.tile([128, 1], f32, tag=f"mean2_{zi}")
                    var2 = small.tile([128, 1], f32, tag=f"var2_{zi}")
                    tmp2 = small.tile([128, 1], f32, tag=f"tmp2_{zi}")
                    nc.vector.tensor_scalar_mul(
                        mean2[:], stats2_g[:, 2 * zi:2 * zi + 1], 1.0 / N)
                    nc.vector.tensor_scalar_mul(
                        var2[:], stats2_g[:, 2 * zi + 1:2 * zi + 2], 1.0 / N)
                    nc.vector.tensor_mul(tmp2[:], mean2[:], mean2[:])
                    nc.vector.tensor_sub(var2[:], var2[:], tmp2[:])
                    nc.scalar.activation(tmp2[:], var2[:], AF.Sqrt, bias=eps_sb[:])
                    nc.vector.reciprocal(istd2[:, zi:zi + 1], tmp2[:])
                    nc.vector.tensor_mul(tmp2[:], mean2[:], istd2[:, zi:zi + 1])
                    nc.vector.tensor_scalar_mul(shift2[:, zi:zi + 1], tmp2[:],
                                                -1.0)

                    # ---- P = BN2(U); qhat = P/||P||; AllGather this z ----
                    ph = php.tile([128, R], f32, tag="ph")
                    nc.vector.tensor_scalar(
                        ph[:], un[:], istd2[:, zi:zi + 1],
                        shift2[:, zi:zi + 1], ALU.mult, ALU.add)
                    sq = sqp.tile([128, R], f32r, tag="sq")
                    nc.scalar.activation(sq[:], ph[:], AF.Square)
                    n2 = ppn.tile([1, R], f32, tag="psn")
                    nc.tensor.matmul(n2[:], ones_k[:], sq[:], start=True,
                                     stop=True)
                    nrm = nrmp.tile([1, R], f32, tag="nrm")
                    nc.scalar.activation(nrm[:], n2[:], AF.Sqrt)
                    rinv = nrmp.tile([1, R], f32r, tag="rinv")
                    with nc.allow_low_precision(
                            reason="f32r rounding of 1/||p|| is intentional"):
                        nc.vector.reciprocal(rinv[:], nrm[:])
                    rb = ppb.tile([128, R], f32, tag="psb")
                    nc.tensor.matmul(rb[:], ones_m[:], rinv[:], start=True,
                                     stop=True)
                    nc.vector.tensor_mul(qloc[:, zsl], ph[:], rb[:])

                    nc.sync.dma_start(ag_in[zi].ap().bitcast(f32r)[:],
                                      qloc[:, zsl])
                    nc.gpsimd.collective_compute(
                        "AllGather", ALU.bypass, replica_groups=rg,
                        ins=[ag_in[zi].ap()], outs=[ag_out[zi].ap()])

            # phase 1-3 pools released here
            if dbg:
                nc.sync.dma_start(dbg_t["qloc"].ap().bitcast(f32r)[:], qloc[:])

            with tc.tile_pool(name="qf", bufs=1) as qfp, \
                 tc.tile_pool(name="stage", bufs=4) as stp, \
                 tc.tile_pool(name="ps5", bufs=3, space="PSUM") as pp5:
                # qfull columns: 4096*zb + 512*cb + i  (global column order)
                qfull = qfp.tile([128, 2 * N], f32r, tag="qf")
                for zb in range(2):
                    for cb in range(NCORES):
                        nc.sync.dma_start(
                            qfull[:, N * zb + R * cb:N * zb + R * (cb + 1)],
                            ag_out[zb].ap().bitcast(f32r)[128 * cb:
                                                          128 * (cb + 1), :])
                if dbg:
                    nc.sync.dma_start(dbg_t["qfull"].ap().bitcast(f32r)[:],
                                      qfull[:])

                # ---- phase 5: row-block of the 2N x 2N score matrix ----
                # loop zb-major so z1 columns (needing only AllGather-z1)
                # complete while the z2 tail is still running
                for zb in range(2):
                    for t in range(8):
                        lhs = qloc[:, 128 * t:128 * (t + 1)]
                        out_dram = out_top if t < 4 else out_bot
                        row0 = 128 * (t % 4)
                        for h in range(2):   # two 2048-col staging chunks
                            ob = stp.tile([128, N // 2], f32, tag="stage")
                            for q in range(4):
                                cb = 4 * h + q
                                pss = pp5.tile([128, R], f32, tag="ps5")
                                nc.tensor.matmul(
                                    pss[:], lhs,
                                    qfull[:, N * zb + R * cb:
                                          N * zb + R * (cb + 1)],
                                    start=True, stop=True)
                                osl = ob[:, R * q:R * (q + 1)]
                                if q % 2 == 0:
                                    nc.vector.tensor_scalar_mul(
                                        osl, pss[:], TEMP_SCALE)
                                else:
                                    nc.scalar.mul(osl, pss[:], TEMP_SCALE)
                            gcol = N * zb + (N // 2) * h
                            nc.sync.dma_start(
                                out_dram.ap()[row0:row0 + 128,
                                              gcol:gcol + N // 2], ob[:])

    nc.compile()
    return nc


def _get_nc():
    if "nc" not in _CACHE:
        _CACHE["nc"] = _build()
    return _CACHE["nc"]


def _make_in_maps(z1, z2, W1, gamma1, beta1, W2):
    W1T = np.ascontiguousarray(W1.T)                       # [k, j]
    W1c = np.ascontiguousarray(
        W1T.reshape(D, KT, 128).transpose(1, 0, 2))        # [j, k, 128]
    W2c = np.ascontiguousarray(W2.T)                       # [D, 128]
    gb = np.concatenate([gamma1.reshape(KT, 128).T,
                         beta1.reshape(KT, 128).T], axis=1).astype(np.float32)
    gb = np.ascontiguousarray(gb)                          # [128, 32]

    in_maps = []
    for c in range(NCORES):
        rs = slice(R * c, R * (c + 1))
        zTc = np.concatenate([z1[rs].T, z2[rs].T], axis=1)  # [D, 1024]
        in_maps.append({
            "zT": np.ascontiguousarray(zTc),
            "W1c": W1c, "W2c": W2c, "gb": gb,
        })
    return in_maps


def kernel(z1, z2, W1, gamma1, beta1, W2):
    z1 = np.asarray(z1, dtype=np.float32)
    z2 = np.asarray(z2, dtype=np.float32)
    W1 = np.asarray(W1, dtype=np.float32)
    W2 = np.asarray(W2, dtype=np.float32)
    gamma1 = np.asarray(gamma1, dtype=np.float32)
    beta1 = np.asarray(beta1, dtype=np.float32)

    in_maps = _make_in_maps(z1, z2, W1, gamma1, beta1, W2)
    nc = _get_nc()
    res = run_bass_kernel_spmd(nc, in_maps, core_ids=list(range(NCORES)))

    scores = np.empty((2 * N, 2 * N), dtype=np.float32)
    for c in range(NCORES):
        scores[R * c:R * (c + 1), :] = res.results[c]["out_top"]
        scores[N + R * c:N + R * (c + 1), :] = res.results[c]["out_bot"]
    idx = np.arange(N)
    scores[idx, idx] = -np.inf
    scores[N + idx, N + idx] = -np.inf
    targets = np.arange(2 * N, dtype=np.int32)
    return scores, targets


# revision 9
# speedup vs baseline: 1.0774x; 1.0774x over previous
"""Trainium2 Bass kernel for nn_LinearCritic (SimCLR-style loss scores).

Pipeline (reference): for each of z1,z2 [4096,2048]:
  X = z @ W1.T ; X = BN(X)*gamma+beta ; Y = relu(X) ; U = Y @ W2.T ; P = BN(U)
then cosine-similarity blocks between normalized projections form a
[8192, 8192] score matrix (diag of s00/s11 = -inf), targets = arange.

Sharding: batch rows split across 8 cores (512 rows of z1 + 512 of z2 each).
BatchNorm batch statistics are global -> tiny AllReduces; the 128-d
normalized projections are AllGathered; each core then computes its
1024-row block of the output (memory-bound: 32 MB of the 256 MB output).

Engine queues are in-order, so the program is emitted in the intended
per-engine execution order: z1's post-GEMM1 chain (BN coeffs, relu, GEMM2,
norms, AllGather) is interleaved INTO z2's GEMM1 j-loop so the collective
latencies hide under PE work, and all collective bounce DMAs live on the
GpSimd queue so they never block the bulk-DMA (Sync) queue. Matmuls run in
float32r (tf32-like, 11-bit mantissa); everything else is fp32.
"""
import sys

sys.path.insert(0, "/opt/trn_rl_repo")

import numpy as np

import concourse.bass as bass
import concourse.bacc as bacc
import concourse.mybir as mybir
import concourse.tile as tile
from concourse.bass_utils import run_bass_kernel_spmd

NCORES = 8
N = 4096          # batch rows per z tensor
D = 2048          # hidden dim
P = 128           # projection dim
R = N // NCORES   # 512 rows per core per z
KT = D // 128     # 16 k/j tiles
TEMP_SCALE = 2.0  # 1/TEMPERATURE
BN_EPS = 1e-5

f32 = mybir.dt.float32
f32r = mybir.dt.float32r

_CACHE = {}


def _build(dbg=False):
    nc = bacc.Bacc("TRN2", target_bir_lowering=False, debug=False,
                   num_devices=NCORES)

    zT = nc.dram_tensor("zT", [D, 2 * R], f32r, kind="ExternalInput")
    W1c = nc.dram_tensor("W1c", [KT, D, 128], f32r, kind="ExternalInput")
    W2c = nc.dram_tensor("W2c", [D, 128], f32r, kind="ExternalInput")
    gb = nc.dram_tensor("gb", [128, 2 * KT], f32, kind="ExternalInput")

    out_top = nc.dram_tensor("out_top", [R, 2 * N], f32, kind="ExternalOutput")
    out_bot = nc.dram_tensor("out_bot", [R, 2 * N], f32, kind="ExternalOutput")

    ar1_in = [nc.dram_tensor(f"ar1_in{z}", [128, 2 * KT], f32, kind="Internal")
              for z in range(2)]
    ar1_out = [nc.dram_tensor(f"ar1_out{z}", [128, 2 * KT], f32,
                              kind="Internal", addr_space="Shared")
               for z in range(2)]
    ar2_in = [nc.dram_tensor(f"ar2_in{z}", [128, 2], f32, kind="Internal")
              for z in range(2)]
    ar2_out = [nc.dram_tensor(f"ar2_out{z}", [128, 2], f32,
                              kind="Internal", addr_space="Shared")
               for z in range(2)]
    ag_in = [nc.dram_tensor(f"ag_in{z}", [128, R], f32, kind="Internal")
             for z in range(2)]
    ag_out = [nc.dram_tensor(f"ag_out{z}", [NCORES * 128, R], f32,
                             kind="Internal", addr_space="Shared")
              for z in range(2)]

    dbg_t = {}
    if dbg:
        dbg_t["qloc"] = nc.dram_tensor("dbg_qloc", [128, 2 * R], f32,
                                       kind="ExternalOutput")
        dbg_t["qfull"] = nc.dram_tensor("dbg_qfull", [128, 2 * N], f32,
                                        kind="ExternalOutput")

    rg = [list(range(NCORES))]
    AF = mybir.ActivationFunctionType
    ALU = mybir.AluOpType

    with tile.TileContext(nc) as tc:
        with tc.tile_pool(name="small", bufs=1) as small, \
             tc.tile_pool(name="persist", bufs=1) as persist, \
             tc.tile_pool(name="uu", bufs=2) as up, \
             tc.tile_pool(name="ph", bufs=2) as php, \
             tc.tile_pool(name="sq", bufs=2) as sqp, \
             tc.tile_pool(name="nrm", bufs=2) as nrmp, \
             tc.tile_pool(name="psn", bufs=1, space="PSUM") as ppn, \
             tc.tile_pool(name="psb", bufs=1, space="PSUM") as ppb:

            stats1 = small.tile([128, 4 * KT], f32, tag="stats1")
            stats1_g = small.tile([128, 4 * KT], f32, tag="stats1g")
            gb_sb = small.tile([128, 2 * KT], f32, tag="gb")
            scale1 = small.tile([128, 2 * KT], f32, tag="scale1")
            shift1 = small.tile([128, 2 * KT], f32, tag="shift1")
            stats2 = small.tile([128, 4], f32, tag="stats2")
            stats2_g = small.tile([128, 4], f32, tag="stats2g")
            istd2 = small.tile([128, 2], f32, tag="istd2")
            shift2 = small.tile([128, 2], f32, tag="shift2")
            ones_k = small.tile([128, 1], f32r, tag="ones_k")
            ones_m = small.tile([1, 128], f32r, tag="ones_m")
            qloc = persist.tile([128, 2 * R], f32r, tag="qloc")
            eps_sb = small.tile([128, 1], f32, tag="eps")
            ones_k32 = small.tile([128, 1], f32, tag="ones_k32")
            ones_m32 = small.tile([1, 128], f32, tag="ones_m32")

            nc.sync.dma_start(gb_sb[:], gb.ap()[:])
            nc.vector.memset(ones_k32[:], 1.0)
            nc.vector.memset(ones_m32[:], 1.0)
            nc.vector.tensor_copy(ones_k[:], ones_k32[:])
            nc.vector.tensor_copy(ones_m[:], ones_m32[:])
            nc.vector.memset(eps_sb[:], BN_EPS)

            xs = {}
            ys = {}
            us = {}

            # ---------- emission helpers (each emits a small chunk) ----------
            def emit_ar1(zi):
                """BN1-stats AllReduce for z[zi]; bounce DMAs on GpSimd."""
                sl = slice(2 * KT * zi, 2 * KT * (zi + 1))
                nc.gpsimd.dma_start(ar1_in[zi].ap()[:], stats1[:, sl])
                nc.gpsimd.collective_compute(
                    "AllReduce", ALU.add, replica_groups=rg,
                    ins=[ar1_in[zi].ap()], outs=[ar1_out[zi].ap()])
                nc.gpsimd.dma_start(stats1_g[:, sl], ar1_out[zi].ap()[:])

            def emit_coef1(zi):
                csl = slice(KT * zi, KT * (zi + 1))
                mean1 = small.tile([128, KT], f32, tag=f"mean1_{zi}")
                var1 = small.tile([128, KT], f32, tag=f"var1_{zi}")
                tmp1 = small.tile([128, KT], f32, tag=f"tmp1_{zi}")
                nc.vector.tensor_scalar_mul(
                    mean1[:], stats1_g[:, 2 * KT * zi:2 * KT * zi + KT], 1.0 / N)
                nc.vector.tensor_scalar_mul(
                    var1[:], stats1_g[:, 2 * KT * zi + KT:2 * KT * (zi + 1)],
                    1.0 / N)
                nc.vector.tensor_mul(tmp1[:], mean1[:], mean1[:])
                nc.vector.tensor_sub(var1[:], var1[:], tmp1[:])
                nc.scalar.activation(tmp1[:], var1[:], AF.Sqrt, bias=eps_sb[:])
                nc.vector.reciprocal(var1[:], tmp1[:])  # var1 := istd
                nc.vector.tensor_mul(scale1[:, csl], var1[:], gb_sb[:, 0:KT])
                nc.vector.tensor_mul(tmp1[:], mean1[:], scale1[:, csl])
                nc.vector.tensor_sub(shift1[:, csl], gb_sb[:, KT:2 * KT], tmp1[:])

            def emit_relu(zi, j):
                yj = yp.tile([128, R], f32r, tag="yy")
                ys[(zi, j)] = yj
                nc.scalar.activation(
                    yj[:], xs[(zi, j)][:], AF.Relu,
                    scale=scale1[:, KT * zi + j:KT * zi + j + 1],
                    bias=shift1[:, KT * zi + j:KT * zi + j + 1])

            def emit_gemm2(zi):
                psu = pp13.tile([128, R], f32, tag="ps13")
                for j in range(KT):
                    nc.tensor.matmul(
                        psu[:], w2sb[:, j, :], ys[(zi, j)][:],
                        start=(j == 0), stop=(j == KT - 1))
                un = up.tile([128, R], f32, tag="uu")
                us[zi] = un
                nc.vector.tensor_scalar(
                    un[:], psu[:], 0.0, 0.0, ALU.add, ALU.add,
                    accum_out=stats2[:, 2 * zi:2 * zi + 1])
                scr = scrp.tile([128, R], f32, tag="scr")
                nc.scalar.activation(
                    scr[:], psu[:], AF.Square,
                    accum_out=stats2[:, 2 * zi + 1:2 * zi + 2])

            def emit_ar2(zi):
                nc.gpsimd.dma_start(ar2_in[zi].ap()[:],
                                    stats2[:, 2 * zi:2 * (zi + 1)])
                nc.gpsimd.collective_compute(
                    "AllReduce", ALU.add, replica_groups=rg,
                    ins=[ar2_in[zi].ap()], outs=[ar2_out[zi].ap()])
                nc.gpsimd.dma_start(stats2_g[:, 2 * zi:2 * (zi + 1)],
                                    ar2_out[zi].ap()[:])

            def emit_coef2(zi):
                mean2 = small.tile([128, 1], f32, tag=f"mean2_{zi}")
                var2 = small.tile([128, 1], f32, tag=f"var2_{zi}")
                tmp2 = small.tile([128, 1], f32, tag=f"tmp2_{zi}")
                nc.vector.tensor_scalar_mul(
                    mean2[:], stats2_g[:, 2 * zi:2 * zi + 1], 1.0 / N)
                nc.vector.tensor_scalar_mul(
                    var2[:], stats2_g[:, 2 * zi + 1:2 * zi + 2], 1.0 / N)
                nc.vector.tensor_mul(tmp2[:], mean2[:], mean2[:])
                nc.vector.tensor_sub(var2[:], var2[:], tmp2[:])
                nc.scalar.activation(tmp2[:], var2[:], AF.Sqrt, bias=eps_sb[:])
                nc.vector.reciprocal(istd2[:, zi:zi + 1], tmp2[:])
                nc.vector.tensor_mul(tmp2[:], mean2[:], istd2[:, zi:zi + 1])
                nc.vector.tensor_scalar_mul(shift2[:, zi:zi + 1], tmp2[:], -1.0)

            def emit_phasec(zi):
                """P = BN2(U); qhat = P/||P||; stage + AllGather this z."""
                zsl = slice(R * zi, R * (zi + 1))
                ph = php.tile([128, R], f32, tag="ph")
                nc.vector.tensor_scalar(
                    ph[:], us[zi][:], istd2[:, zi:zi + 1],
                    shift2[:, zi:zi + 1], ALU.mult, ALU.add)
                sq = sqp.tile([128, R], f32r, tag="sq")
                nc.scalar.activation(sq[:], ph[:], AF.Square)
                n2 = ppn.tile([1, R], f32, tag="psn")
                nc.tensor.matmul(n2[:], ones_k[:], sq[:], start=True, stop=True)
                nrm = nrmp.tile([1, R], f32, tag="nrm")
                nc.scalar.activation(nrm[:], n2[:], AF.Sqrt)
                rinv = nrmp.tile([1, R], f32r, tag="rinv")
                with nc.allow_low_precision(
                        reason="f32r rounding of 1/||p|| is intentional"):
                    nc.vector.reciprocal(rinv[:], nrm[:])
                rb = ppb.tile([128, R], f32, tag="psb")
                nc.tensor.matmul(rb[:], ones_m[:], rinv[:], start=True, stop=True)
                nc.vector.tensor_mul(qloc[:, zsl], ph[:], rb[:])
                nc.gpsimd.dma_start(ag_in[zi].ap().bitcast(f32r)[:], qloc[:, zsl])
                nc.gpsimd.collective_compute(
                    "AllGather", ALU.bypass, replica_groups=rg,
                    ins=[ag_in[zi].ap()], outs=[ag_out[zi].ap()])

            def emit_gemm1_j(zi, j, w1j):
                zsl = slice(R * zi, R * (zi + 1))
                xj = xp.tile([128, R], f32, tag="xx")
                xs[(zi, j)] = xj
                ps = pp13.tile([128, R], f32, tag="ps13")
                for k in range(KT):
                    nc.tensor.matmul(
                        ps[:], w1j[:, k, :], ztile[:, k, zsl],
                        start=(k == 0), stop=(k == KT - 1))
                nc.vector.tensor_scalar(
                    xj[:], ps[:], 0.0, 0.0, ALU.add, ALU.add,
                    accum_out=stats1[:, 2 * KT * zi + j:2 * KT * zi + j + 1])
                scr = scrp.tile([128, R], f32, tag="scr")
                nc.scalar.activation(
                    scr[:], ps[:], AF.Square,
                    accum_out=stats1[:, 2 * KT * zi + KT + j:
                                     2 * KT * zi + KT + j + 1])

            # ---------------- main emission schedule ----------------
            with tc.tile_pool(name="w1", bufs=2) as w1p, \
                 tc.tile_pool(name="ztp", bufs=1) as ztp, \
                 tc.tile_pool(name="xx", bufs=22) as xp, \
                 tc.tile_pool(name="yy", bufs=16) as yp, \
                 tc.tile_pool(name="w2", bufs=1) as w2p, \
                 tc.tile_pool(name="scr", bufs=2) as scrp, \
                 tc.tile_pool(name="ps13", bufs=3, space="PSUM") as pp13:

                # first weight block before the big z load, so PE can start
                # as soon as zT lands
                w1_first = w1p.tile([128, KT, 128], f32r, tag="w1")
                nc.sync.dma_start(
                    w1_first[:], W1c.ap()[0].rearrange("(k p) c -> p k c", p=128))
                ztile = ztp.tile([128, KT, 2 * R], f32r, tag="zt")
                nc.sync.dma_start(
                    ztile[:], zT.ap().rearrange("(k p) n -> p k n", p=128))
                w2sb = w2p.tile([128, KT, 128], f32r, tag="w2")
                nc.sync.dma_start(
                    w2sb[:], W2c.ap().rearrange("(k p) c -> p k c", p=128))

                # GEMM1 for z1
                for j in range(KT):
                    if j == 0:
                        w1j = w1_first
                    else:
                        w1j = w1p.tile([128, KT, 128], f32r, tag="w1")
                        nc.sync.dma_start(
                            w1j[:],
                            W1c.ap()[j].rearrange("(k p) c -> p k c", p=128))
                    emit_gemm1_j(0, j, w1j)

                emit_ar1(0)  # hides under z2's GEMM1

                # GEMM1 for z2, with z1's dependent chain interleaved
                for j in range(KT):
                    w1j = w1p.tile([128, KT, 128], f32r, tag="w1")
                    nc.sync.dma_start(
                        w1j[:], W1c.ap()[j].rearrange("(k p) c -> p k c", p=128))
                    emit_gemm1_j(1, j, w1j)
                    if j == 1:
                        emit_coef1(0)
                    if 2 <= j <= 9:
                        emit_relu(0, 2 * (j - 2))
                        emit_relu(0, 2 * (j - 2) + 1)
                    if j == 10:
                        emit_gemm2(0)
                    if j == 11:
                        emit_ar2(0)
                    if j == 14:
                        emit_coef2(0)

                emit_ar1(1)       # z2 BN1 stats reduce
                emit_phasec(0)    # z1 normalize + AllGather (AR2-z1 done)

                # z2 dependent chain (waits on AR1-z2; PE meanwhile free for
                # early phase-5 blocks emitted below)
                emit_coef1(1)
                for j in range(KT):
                    emit_relu(1, j)
                emit_gemm2(1)
                emit_ar2(1)
                emit_coef2(1)
                emit_phasec(1)

            # phase 1-3 pools released here
            if dbg:
                nc.sync.dma_start(dbg_t["qloc"].ap().bitcast(f32r)[:], qloc[:])

            with tc.tile_pool(name="qf", bufs=1) as qfp, \
                 tc.tile_pool(name="stage", bufs=4) as stp, \
                 tc.tile_pool(name="ps5", bufs=3, space="PSUM") as pp5:
                # qfull columns: 4096*zb + 512*cb + i  (global column order)
                qfull = qfp.tile([128, 2 * N], f32r, tag="qf")

                def emit_qload(zb):
                    for cb in range(NCORES):
                        nc.sync.dma_start(
                            qfull[:, N * zb + R * cb:N * zb + R * (cb + 1)],
                            ag_out[zb].ap().bitcast(f32r)[128 * cb:
                                                          128 * (cb + 1), :])

                def emit_p5(zb, trange):
                    for t in trange:
                        lhs = qloc[:, 128 * t:128 * (t + 1)]
                        out_dram = out_top if t < 4 else out_bot
                        row0 = 128 * (t % 4)
                        for h in range(2):   # two 2048-col staging chunks
                            ob = stp.tile([128, N // 2], f32, tag="stage")
                            for q in range(4):
                                cb = 4 * h + q
                                pss = pp5.tile([128, R], f32, tag="ps5")
                                nc.tensor.matmul(
                                    pss[:], lhs,
                                    qfull[:, N * zb + R * cb:
                                          N * zb + R * (cb + 1)],
                                    start=True, stop=True)
                                osl = ob[:, R * q:R * (q + 1)]
                                if q % 2 == 0:
                                    nc.vector.tensor_scalar_mul(
                                        osl, pss[:], TEMP_SCALE)
                                else:
                                    nc.scalar.mul(osl, pss[:], TEMP_SCALE)
                            gcol = N * zb + (N // 2) * h
                            nc.sync.dma_start(
                                out_dram.ap()[row0:row0 + 128,
                                              gcol:gcol + N // 2], ob[:])

                emit_qload(0)
                if dbg:
                    nc.sync.dma_start(dbg_t["qfull"].ap().bitcast(f32r)
                                      [:, 0:N], qfull[:, 0:N])
                emit_p5(0, range(0, 4))     # z1 rows x z1 cols: earliest ready
                emit_p5(0, range(4, 8))     # z2 rows x z1 cols
                emit_qload(1)
                if dbg:
                    nc.sync.dma_start(dbg_t["qfull"].ap().bitcast(f32r)
                                      [:, N:2 * N], qfull[:, N:2 * N])
                emit_p5(1, range(0, 8))     # all rows x z2 cols

    nc.compile()
    return nc


def _get_nc():
    if "nc" not in _CACHE:
        _CACHE["nc"] = _build()
    return _CACHE["nc"]


def _make_in_maps(z1, z2, W1, gamma1, beta1, W2):
    W1T = np.ascontiguousarray(W1.T)                       # [k, j]
    W1c = np.ascontiguousarray(
        W1T.reshape(D, KT, 128).transpose(1, 0, 2))        # [j, k, 128]
    W2c = np.ascontiguousarray(W2.T)                       # [D, 128]
    gb = np.concatenate([gamma1.reshape(KT, 128).T,
                         beta1.reshape(KT, 128).T], axis=1).astype(np.float32)
    gb = np.ascontiguousarray(gb)                          # [128, 32]

    in_maps = []
    for c in range(NCORES):
        rs = slice(R * c, R * (c + 1))
        zTc = np.concatenate([z1[rs].T, z2[rs].T], axis=1)  # [D, 1024]
        in_maps.append({
            "zT": np.ascontiguousarray(zTc),
            "W1c": W1c, "W2c": W2c, "gb": gb,
        })
    return in_maps


def kernel(z1, z2, W1, gamma1, beta1, W2):
    z1 = np.asarray(z1, dtype=np.float32)
    z2 = np.asarray(z2, dtype=np.float32)
    W1 = np.asarray(W1, dtype=np.float32)
    W2 = np.asarray(W2, dtype=np.float32)
    gamma1 = np.asarray(gamma1, dtype=np.float32)
    beta1 = np.asarray(beta1, dtype=np.float32)

    in_maps = _make_in_maps(z1, z2, W1, gamma1, beta1, W2)
    nc = _get_nc()
    res = run_bass_kernel_spmd(nc, in_maps, core_ids=list(range(NCORES)))

    scores = np.empty((2 * N, 2 * N), dtype=np.float32)
    for c in range(NCORES):
        scores[R * c:R * (c + 1), :] = res.results[c]["out_top"]
        scores[N + R * c:N + R * (c + 1), :] = res.results[c]["out_bot"]
    idx = np.arange(N)
    scores[idx, idx] = -np.inf
    scores[N + idx, N + idx] = -np.inf
    targets = np.arange(2 * N, dtype=np.int32)
    return scores, targets


# revision 10
# speedup vs baseline: 1.1229x; 1.0422x over previous
"""Trainium2 Bass kernel for nn_LinearCritic (SimCLR-style loss scores).

Pipeline (reference): for each of z1,z2 [4096,2048]:
  X = z @ W1.T ; X = BN(X)*gamma+beta ; Y = relu(X) ; U = Y @ W2.T ; P = BN(U)
then cosine-similarity blocks between normalized projections form a
[8192, 8192] score matrix (diag of s00/s11 = -inf), targets = arange.

Sharding: batch rows split across 8 cores (512 rows of z1 + 512 of z2 each).
BatchNorm batch statistics are global -> tiny AllReduces; the 128-d
normalized projections are AllGathered; each core then computes its
1024-row block of the output (memory-bound: 32 MB of the 256 MB output).

Engine queues are in-order, so the program is emitted in the intended
per-engine execution order: z1's post-GEMM1 chain (BN coeffs, relu, GEMM2,
norms, AllGather) is interleaved INTO z2's GEMM1 j-loop so the collective
latencies hide under PE work, and all collective bounce DMAs live on the
GpSimd queue so they never block the bulk-DMA (Sync) queue. Matmuls run in
float32r (tf32-like, 11-bit mantissa); everything else is fp32.
"""
import sys

sys.path.insert(0, "/opt/trn_rl_repo")

import numpy as np

import concourse.bass as bass
import concourse.bacc as bacc
import concourse.mybir as mybir
import concourse.tile as tile
from concourse.bass_utils import run_bass_kernel_spmd

NCORES = 8
N = 4096          # batch rows per z tensor
D = 2048          # hidden dim
P = 128           # projection dim
R = N // NCORES   # 512 rows per core per z
KT = D // 128     # 16 k/j tiles
TEMP_SCALE = 2.0  # 1/TEMPERATURE
BN_EPS = 1e-5

f32 = mybir.dt.float32
f32r = mybir.dt.float32r

_CACHE = {}


def _build(dbg=False):
    nc = bacc.Bacc("TRN2", target_bir_lowering=False, debug=False,
                   num_devices=NCORES)

    zT = nc.dram_tensor("zT", [D, 2 * R], f32r, kind="ExternalInput")
    W1c = nc.dram_tensor("W1c", [KT, D, 128], f32r, kind="ExternalInput")
    W2c = nc.dram_tensor("W2c", [D, 128], f32r, kind="ExternalInput")
    gb = nc.dram_tensor("gb", [128, 2 * KT], f32, kind="ExternalInput")

    out_top = nc.dram_tensor("out_top", [R, 2 * N], f32, kind="ExternalOutput")
    out_bot = nc.dram_tensor("out_bot", [R, 2 * N], f32, kind="ExternalOutput")

    ar1_in = [nc.dram_tensor(f"ar1_in{z}", [128, 2 * KT], f32, kind="Internal")
              for z in range(2)]
    ar1_out = [nc.dram_tensor(f"ar1_out{z}", [128, 2 * KT], f32,
                              kind="Internal", addr_space="Shared")
               for z in range(2)]
    ar2_in = [nc.dram_tensor(f"ar2_in{z}", [128, 2], f32, kind="Internal")
              for z in range(2)]
    ar2_out = [nc.dram_tensor(f"ar2_out{z}", [128, 2], f32,
                              kind="Internal", addr_space="Shared")
               for z in range(2)]
    ag_in = [nc.dram_tensor(f"ag_in{z}", [128, R], f32, kind="Internal")
             for z in range(2)]
    ag_out = [nc.dram_tensor(f"ag_out{z}", [NCORES * 128, R], f32,
                             kind="Internal", addr_space="Shared")
              for z in range(2)]

    dbg_t = {}
    if dbg:
        dbg_t["qloc"] = nc.dram_tensor("dbg_qloc", [128, 2 * R], f32,
                                       kind="ExternalOutput")
        dbg_t["qfull"] = nc.dram_tensor("dbg_qfull", [128, 2 * N], f32,
                                        kind="ExternalOutput")

    rg = [list(range(NCORES))]
    AF = mybir.ActivationFunctionType
    ALU = mybir.AluOpType

    with tile.TileContext(nc) as tc:
        with tc.tile_pool(name="small", bufs=1) as small, \
             tc.tile_pool(name="persist", bufs=1) as persist, \
             tc.tile_pool(name="uu", bufs=2) as up, \
             tc.tile_pool(name="ph", bufs=2) as php, \
             tc.tile_pool(name="sq", bufs=2) as sqp, \
             tc.tile_pool(name="nrm", bufs=2) as nrmp, \
             tc.tile_pool(name="psn", bufs=1, space="PSUM") as ppn, \
             tc.tile_pool(name="psb", bufs=1, space="PSUM") as ppb:

            stats1 = small.tile([128, 4 * KT], f32, tag="stats1")
            stats1_g = small.tile([128, 4 * KT], f32, tag="stats1g")
            gb_sb = small.tile([128, 2 * KT], f32, tag="gb")
            scale1 = small.tile([128, 2 * KT], f32, tag="scale1")
            shift1 = small.tile([128, 2 * KT], f32, tag="shift1")
            stats2 = small.tile([128, 4], f32, tag="stats2")
            stats2_g = small.tile([128, 4], f32, tag="stats2g")
            istd2 = small.tile([128, 2], f32, tag="istd2")
            shift2 = small.tile([128, 2], f32, tag="shift2")
            ones_k = small.tile([128, 1], f32r, tag="ones_k")
            ones_m = small.tile([1, 128], f32r, tag="ones_m")
            qloc = persist.tile([128, 2 * R], f32r, tag="qloc")
            eps_sb = small.tile([128, 1], f32, tag="eps")
            ones_k32 = small.tile([128, 1], f32, tag="ones_k32")
            ones_m32 = small.tile([1, 128], f32, tag="ones_m32")

            nc.sync.dma_start(gb_sb[:], gb.ap()[:])
            nc.vector.memset(ones_k32[:], 1.0)
            nc.vector.memset(ones_m32[:], 1.0)
            nc.vector.tensor_copy(ones_k[:], ones_k32[:])
            nc.vector.tensor_copy(ones_m[:], ones_m32[:])
            nc.vector.memset(eps_sb[:], BN_EPS)

            xs = {}
            ys = {}
            us = {}

            # ---------- emission helpers (each emits a small chunk) ----------
            def emit_ar1(zi):
                """BN1-stats AllReduce for z[zi]; bounce DMAs on GpSimd."""
                sl = slice(2 * KT * zi, 2 * KT * (zi + 1))
                nc.gpsimd.dma_start(ar1_in[zi].ap()[:], stats1[:, sl])
                nc.gpsimd.collective_compute(
                    "AllReduce", ALU.add, replica_groups=rg,
                    ins=[ar1_in[zi].ap()], outs=[ar1_out[zi].ap()])
                nc.gpsimd.dma_start(stats1_g[:, sl], ar1_out[zi].ap()[:])

            def emit_coef1(zi):
                csl = slice(KT * zi, KT * (zi + 1))
                mean1 = small.tile([128, KT], f32, tag=f"mean1_{zi}")
                var1 = small.tile([128, KT], f32, tag=f"var1_{zi}")
                tmp1 = small.tile([128, KT], f32, tag=f"tmp1_{zi}")
                nc.vector.tensor_scalar_mul(
                    mean1[:], stats1_g[:, 2 * KT * zi:2 * KT * zi + KT], 1.0 / N)
                nc.vector.tensor_scalar_mul(
                    var1[:], stats1_g[:, 2 * KT * zi + KT:2 * KT * (zi + 1)],
                    1.0 / N)
                nc.vector.tensor_mul(tmp1[:], mean1[:], mean1[:])
                nc.vector.tensor_sub(var1[:], var1[:], tmp1[:])
                nc.scalar.activation(tmp1[:], var1[:], AF.Sqrt, bias=eps_sb[:])
                nc.vector.reciprocal(var1[:], tmp1[:])  # var1 := istd
                nc.vector.tensor_mul(scale1[:, csl], var1[:], gb_sb[:, 0:KT])
                nc.vector.tensor_mul(tmp1[:], mean1[:], scale1[:, csl])
                nc.vector.tensor_sub(shift1[:, csl], gb_sb[:, KT:2 * KT], tmp1[:])

            def emit_relu(zi, j, engine="act"):
                yj = yp.tile([128, R], f32r, tag="yy")
                ys[(zi, j)] = yj
                if engine == "act":
                    nc.scalar.activation(
                        yj[:], xs[(zi, j)][:], AF.Relu,
                        scale=scale1[:, KT * zi + j:KT * zi + j + 1],
                        bias=shift1[:, KT * zi + j:KT * zi + j + 1])
                else:
                    nc.vector.tensor_scalar(
                        yj[:], xs[(zi, j)][:],
                        scale1[:, KT * zi + j:KT * zi + j + 1],
                        shift1[:, KT * zi + j:KT * zi + j + 1],
                        ALU.mult, ALU.add)
                    nc.vector.tensor_scalar_max(yj[:], yj[:], 0.0)

            def emit_gemm2(zi):
                psu = pp13.tile([128, R], f32, tag="ps13")
                for j in range(KT):
                    nc.tensor.matmul(
                        psu[:], w2sb[:, j, :], ys[(zi, j)][:],
                        start=(j == 0), stop=(j == KT - 1))
                un = up.tile([128, R], f32, tag="uu")
                us[zi] = un
                nc.vector.tensor_scalar(
                    un[:], psu[:], 0.0, 0.0, ALU.add, ALU.add,
                    accum_out=stats2[:, 2 * zi:2 * zi + 1])
                scr = scrp.tile([128, R], f32, tag="scr")
                nc.scalar.activation(
                    scr[:], psu[:], AF.Square,
                    accum_out=stats2[:, 2 * zi + 1:2 * zi + 2])

            def emit_ar2(zi):
                nc.gpsimd.dma_start(ar2_in[zi].ap()[:],
                                    stats2[:, 2 * zi:2 * (zi + 1)])
                nc.gpsimd.collective_compute(
                    "AllReduce", ALU.add, replica_groups=rg,
                    ins=[ar2_in[zi].ap()], outs=[ar2_out[zi].ap()])
                nc.gpsimd.dma_start(stats2_g[:, 2 * zi:2 * (zi + 1)],
                                    ar2_out[zi].ap()[:])

            def emit_coef2(zi):
                mean2 = small.tile([128, 1], f32, tag=f"mean2_{zi}")
                var2 = small.tile([128, 1], f32, tag=f"var2_{zi}")
                tmp2 = small.tile([128, 1], f32, tag=f"tmp2_{zi}")
                nc.vector.tensor_scalar_mul(
                    mean2[:], stats2_g[:, 2 * zi:2 * zi + 1], 1.0 / N)
                nc.vector.tensor_scalar_mul(
                    var2[:], stats2_g[:, 2 * zi + 1:2 * zi + 2], 1.0 / N)
                nc.vector.tensor_mul(tmp2[:], mean2[:], mean2[:])
                nc.vector.tensor_sub(var2[:], var2[:], tmp2[:])
                nc.scalar.activation(tmp2[:], var2[:], AF.Sqrt, bias=eps_sb[:])
                nc.vector.reciprocal(istd2[:, zi:zi + 1], tmp2[:])
                nc.vector.tensor_mul(tmp2[:], mean2[:], istd2[:, zi:zi + 1])
                nc.vector.tensor_scalar_mul(shift2[:, zi:zi + 1], tmp2[:], -1.0)

            def emit_phasec(zi):
                """P = BN2(U); qhat = P/||P||; stage + AllGather this z."""
                zsl = slice(R * zi, R * (zi + 1))
                ph = php.tile([128, R], f32, tag="ph")
                nc.vector.tensor_scalar(
                    ph[:], us[zi][:], istd2[:, zi:zi + 1],
                    shift2[:, zi:zi + 1], ALU.mult, ALU.add)
                sq = sqp.tile([128, R], f32r, tag="sq")
                nc.scalar.activation(sq[:], ph[:], AF.Square)
                n2 = ppn.tile([1, R], f32, tag="psn")
                nc.tensor.matmul(n2[:], ones_k[:], sq[:], start=True, stop=True)
                nrm = nrmp.tile([1, R], f32, tag="nrm")
                nc.scalar.activation(nrm[:], n2[:], AF.Sqrt)
                rinv = nrmp.tile([1, R], f32r, tag="rinv")
                with nc.allow_low_precision(
                        reason="f32r rounding of 1/||p|| is intentional"):
                    nc.vector.reciprocal(rinv[:], nrm[:])
                rb = ppb.tile([128, R], f32, tag="psb")
                nc.tensor.matmul(rb[:], ones_m[:], rinv[:], start=True, stop=True)
                nc.vector.tensor_mul(qloc[:, zsl], ph[:], rb[:])
                nc.gpsimd.dma_start(ag_in[zi].ap().bitcast(f32r)[:], qloc[:, zsl])
                nc.gpsimd.collective_compute(
                    "AllGather", ALU.bypass, replica_groups=rg,
                    ins=[ag_in[zi].ap()], outs=[ag_out[zi].ap()])

            def emit_gemm1_j(zi, j, w1j):
                zsl = slice(R * zi, R * (zi + 1))
                xj = xp.tile([128, R], f32, tag="xx")
                xs[(zi, j)] = xj
                ps = pp13.tile([128, R], f32, tag="ps13")
                for k in range(KT):
                    nc.tensor.matmul(
                        ps[:], w1j[:, k, :], zt_k(k, zsl),
                        start=(k == 0), stop=(k == KT - 1))
                nc.vector.tensor_scalar(
                    xj[:], ps[:], 0.0, 0.0, ALU.add, ALU.add,
                    accum_out=stats1[:, 2 * KT * zi + j:2 * KT * zi + j + 1])
                scr = scrp.tile([128, R], f32, tag="scr")
                nc.scalar.activation(
                    scr[:], ps[:], AF.Square,
                    accum_out=stats1[:, 2 * KT * zi + KT + j:
                                     2 * KT * zi + KT + j + 1])

            # ---------------- main emission schedule ----------------
            with tc.tile_pool(name="w1", bufs=2) as w1p, \
                 tc.tile_pool(name="ztp", bufs=1) as ztp, \
                 tc.tile_pool(name="xx", bufs=22) as xp, \
                 tc.tile_pool(name="yy", bufs=16) as yp, \
                 tc.tile_pool(name="w2", bufs=1) as w2p, \
                 tc.tile_pool(name="scr", bufs=2) as scrp, \
                 tc.tile_pool(name="ps13", bufs=3, space="PSUM") as pp13:

                # first weight block before the big z load, so PE can start
                # as soon as zT lands
                w1_first = w1p.tile([128, KT, 128], f32r, tag="w1")
                nc.sync.dma_start(
                    w1_first[:], W1c.ap()[0].rearrange("(k p) c -> p k c", p=128))
                zta = ztp.tile([128, KT // 2, 2 * R], f32r, tag="zta")
                ztb = ztp.tile([128, KT // 2, 2 * R], f32r, tag="ztb")
                nc.sync.dma_start(
                    zta[:], zT.ap()[0:D // 2].rearrange("(k p) n -> p k n", p=128))
                nc.sync.dma_start(
                    ztb[:], zT.ap()[D // 2:D].rearrange("(k p) n -> p k n", p=128))

                def zt_k(k, zsl):
                    if k < KT // 2:
                        return zta[:, k, zsl]
                    return ztb[:, k - KT // 2, zsl]

                w2sb = w2p.tile([128, KT, 128], f32r, tag="w2")
                nc.sync.dma_start(
                    w2sb[:], W2c.ap().rearrange("(k p) c -> p k c", p=128))

                # GEMM1 for z1
                for j in range(KT):
                    if j == 0:
                        w1j = w1_first
                    else:
                        w1j = w1p.tile([128, KT, 128], f32r, tag="w1")
                        nc.sync.dma_start(
                            w1j[:],
                            W1c.ap()[j].rearrange("(k p) c -> p k c", p=128))
                    emit_gemm1_j(0, j, w1j)

                emit_ar1(0)  # hides under z2's GEMM1

                # GEMM1 for z2, with z1's dependent chain interleaved
                for j in range(KT):
                    w1j = w1p.tile([128, KT, 128], f32r, tag="w1")
                    nc.sync.dma_start(
                        w1j[:], W1c.ap()[j].rearrange("(k p) c -> p k c", p=128))
                    emit_gemm1_j(1, j, w1j)
                    if j == 1:
                        emit_coef1(0)
                    if 2 <= j <= 9:
                        emit_relu(0, 2 * (j - 2))
                        emit_relu(0, 2 * (j - 2) + 1)
                    if j == 10:
                        emit_gemm2(0)
                    if j == 11:
                        emit_ar2(0)
                    if j == 14:
                        emit_coef2(0)

                emit_ar1(1)       # z2 BN1 stats reduce (TOPSP before AG-z1)
                emit_phasec(0)    # z1 normalize + AllGather (AR2-z1 done)

                # z2 dependent chain: split relu across ACT and DVE to
                # shorten the critical path; GEMM2/AR2/phaseC-z2 emitted
                # before phase 5 so the PE and TOPSP queues don't block
                emit_coef1(1)
                for j in range(KT):
                    emit_relu(1, j, engine=("dve" if j % 3 == 2 else "act"))
                emit_gemm2(1)
                emit_ar2(1)
                emit_coef2(1)
                emit_phasec(1)

            # phase 1-3 pools released here
            if dbg:
                nc.sync.dma_start(dbg_t["qloc"].ap().bitcast(f32r)[:], qloc[:])

            with tc.tile_pool(name="qf", bufs=1) as qfp, \
                 tc.tile_pool(name="stage", bufs=4) as stp, \
                 tc.tile_pool(name="ps5", bufs=3, space="PSUM") as pp5:
                # qfull columns: 4096*zb + 512*cb + i  (global column order)
                qfull = qfp.tile([128, 2 * N], f32r, tag="qf")

                def emit_qload(zb):
                    for cb in range(NCORES):
                        nc.sync.dma_start(
                            qfull[:, N * zb + R * cb:N * zb + R * (cb + 1)],
                            ag_out[zb].ap().bitcast(f32r)[128 * cb:
                                                          128 * (cb + 1), :])

                def emit_p5(zb, trange):
                    for t in trange:
                        lhs = qloc[:, 128 * t:128 * (t + 1)]
                        out_dram = out_top if t < 4 else out_bot
                        row0 = 128 * (t % 4)
                        for h in range(2):   # two 2048-col staging chunks
                            ob = stp.tile([128, N // 2], f32, tag="stage")
                            for q in range(4):
                                cb = 4 * h + q
                                pss = pp5.tile([128, R], f32, tag="ps5")
                                nc.tensor.matmul(
                                    pss[:], lhs,
                                    qfull[:, N * zb + R * cb:
                                          N * zb + R * (cb + 1)],
                                    start=True, stop=True)
                                osl = ob[:, R * q:R * (q + 1)]
                                if q % 2 == 0:
                                    nc.vector.tensor_scalar_mul(
                                        osl, pss[:], TEMP_SCALE)
                                else:
                                    nc.scalar.mul(osl, pss[:], TEMP_SCALE)
                            gcol = N * zb + (N // 2) * h
                            nc.sync.dma_start(
                                out_dram.ap()[row0:row0 + 128,
                                              gcol:gcol + N // 2], ob[:])

                emit_qload(0)
                if dbg:
                    nc.sync.dma_start(dbg_t["qfull"].ap().bitcast(f32r)
                                      [:, 0:N], qfull[:, 0:N])
                emit_p5(0, range(0, 4))     # z1 rows x z1 cols: earliest ready
                emit_p5(0, range(4, 8))     # z2 rows x z1 cols
                emit_qload(1)
                if dbg:
                    nc.sync.dma_start(dbg_t["qfull"].ap().bitcast(f32r)
                                      [:, N:2 * N], qfull[:, N:2 * N])
                emit_p5(1, range(0, 8))     # all rows x z2 cols

    nc.compile()
    return nc


def _get_nc():
    if "nc" not in _CACHE:
        _CACHE["nc"] = _build()
    return _CACHE["nc"]


def _make_in_maps(z1, z2, W1, gamma1, beta1, W2):
    W1T = np.ascontiguousarray(W1.T)                       # [k, j]
    W1c = np.ascontiguousarray(
        W1T.reshape(D, KT, 128).transpose(1, 0, 2))        # [j, k, 128]
    W2c = np.ascontiguousarray(W2.T)                       # [D, 128]
    gb = np.concatenate([gamma1.reshape(KT, 128).T,
                         beta1.reshape(KT, 128).T], axis=1).astype(np.float32)
    gb = np.ascontiguousarray(gb)                          # [128, 32]

    in_maps = []
    for c in range(NCORES):
        rs = slice(R * c, R * (c + 1))
        zTc = np.concatenate([z1[rs].T, z2[rs].T], axis=1)  # [D, 1024]
        in_maps.append({
            "zT": np.ascontiguousarray(zTc),
            "W1c": W1c, "W2c": W2c, "gb": gb,
        })
    return in_maps


def kernel(z1, z2, W1, gamma1, beta1, W2):
    z1 = np.asarray(z1, dtype=np.float32)
    z2 = np.asarray(z2, dtype=np.float32)
    W1 = np.asarray(W1, dtype=np.float32)
    W2 = np.asarray(W2, dtype=np.float32)
    gamma1 = np.asarray(gamma1, dtype=np.float32)
    beta1 = np.asarray(beta1, dtype=np.float32)

    in_maps = _make_in_maps(z1, z2, W1, gamma1, beta1, W2)
    nc = _get_nc()
    res = run_bass_kernel_spmd(nc, in_maps, core_ids=list(range(NCORES)))

    scores = np.empty((2 * N, 2 * N), dtype=np.float32)
    for c in range(NCORES):
        scores[R * c:R * (c + 1), :] = res.results[c]["out_top"]
        scores[N + R * c:N + R * (c + 1), :] = res.results[c]["out_bot"]
    idx = np.arange(N)
    scores[idx, idx] = -np.inf
    scores[N + idx, N + idx] = -np.inf
    targets = np.arange(2 * N, dtype=np.int32)
    return scores, targets


# revision 11
# speedup vs baseline: 1.3411x; 1.1943x over previous
"""Trainium2 Bass kernel for nn_LinearCritic (SimCLR-style loss scores).

Pipeline (reference): for each of z1,z2 [4096,2048]:
  X = z @ W1.T ; X = BN(X)*gamma+beta ; Y = relu(X) ; U = Y @ W2.T ; P = BN(U)
then cosine-similarity blocks between normalized projections form a
[8192, 8192] score matrix (diag of s00/s11 = -inf), targets = arange.

Sharding: batch rows split across 8 cores (512 rows of z1 + 512 of z2 each).
BatchNorm batch statistics are global -> tiny AllReduces; the 128-d
normalized projections are AllGathered; each core then computes its
1024-row block of the output (memory-bound: 32 MB of the 256 MB output).

Engine queues are in-order, so the program is emitted in the intended
per-engine execution order: z1's post-GEMM1 chain (BN coeffs, relu, GEMM2,
norms, AllGather) is interleaved INTO z2's GEMM1 j-loop so the collective
latencies hide under PE work, and all collective bounce DMAs live on the
GpSimd queue so they never block the bulk-DMA (Sync) queue. Matmuls run in
float32r (tf32-like, 11-bit mantissa); everything else is fp32.
"""
import sys

sys.path.insert(0, "/opt/trn_rl_repo")

import ml_dtypes
import numpy as np

import concourse.bass as bass
import concourse.bacc as bacc
import concourse.mybir as mybir
import concourse.tile as tile
from concourse.bass_utils import run_bass_kernel_spmd

NCORES = 8
N = 4096          # batch rows per z tensor
D = 2048          # hidden dim
P = 128           # projection dim
R = N // NCORES   # 512 rows per core per z
KT = D // 128     # 16 k/j tiles
TEMP_SCALE = 2.0  # 1/TEMPERATURE
BN_EPS = 1e-5

f32 = mybir.dt.float32
f32r = mybir.dt.float32r
bf16 = mybir.dt.bfloat16

_CACHE = {}


def _build(dbg=False):
    nc = bacc.Bacc("TRN2", target_bir_lowering=False, debug=False,
                   num_devices=NCORES)

    zT = nc.dram_tensor("zT", [D, 2 * R], bf16, kind="ExternalInput")
    W1c = nc.dram_tensor("W1c", [KT, D, 128], bf16, kind="ExternalInput")
    W2c = nc.dram_tensor("W2c", [D, 128], f32r, kind="ExternalInput")
    gb = nc.dram_tensor("gb", [128, 2 * KT], f32, kind="ExternalInput")

    out_top = nc.dram_tensor("out_top", [R, 2 * N], f32, kind="ExternalOutput")
    out_bot = nc.dram_tensor("out_bot", [R, 2 * N], f32, kind="ExternalOutput")

    ar1_in = [nc.dram_tensor(f"ar1_in{z}", [128, 2 * KT], f32, kind="Internal")
              for z in range(2)]
    ar1_out = [nc.dram_tensor(f"ar1_out{z}", [128, 2 * KT], f32,
                              kind="Internal", addr_space="Shared")
               for z in range(2)]
    ar2_in = [nc.dram_tensor(f"ar2_in{z}", [128, 2], f32, kind="Internal")
              for z in range(2)]
    ar2_out = [nc.dram_tensor(f"ar2_out{z}", [128, 2], f32,
                              kind="Internal", addr_space="Shared")
               for z in range(2)]
    ag_in = [nc.dram_tensor(f"ag_in{z}", [128, R], bf16, kind="Internal")
             for z in range(2)]
    ag_out = [nc.dram_tensor(f"ag_out{z}", [NCORES * 128, R], bf16,
                             kind="Internal", addr_space="Shared")
              for z in range(2)]

    dbg_t = {}
    if dbg:
        dbg_t["qloc"] = nc.dram_tensor("dbg_qloc", [128, 2 * R], bf16,
                                       kind="ExternalOutput")
        dbg_t["qfull"] = nc.dram_tensor("dbg_qfull", [128, 2 * N], bf16,
                                        kind="ExternalOutput")

    rg = [list(range(NCORES))]
    AF = mybir.ActivationFunctionType
    ALU = mybir.AluOpType

    with tile.TileContext(nc) as tc:
        with tc.tile_pool(name="small", bufs=1) as small, \
             tc.tile_pool(name="persist", bufs=1) as persist, \
             tc.tile_pool(name="uu", bufs=2) as up, \
             tc.tile_pool(name="ph", bufs=2) as php, \
             tc.tile_pool(name="sq", bufs=2) as sqp, \
             tc.tile_pool(name="nrm", bufs=2) as nrmp, \
             tc.tile_pool(name="psn", bufs=1, space="PSUM") as ppn, \
             tc.tile_pool(name="psb", bufs=1, space="PSUM") as ppb:

            stats1 = small.tile([128, 4 * KT], f32, tag="stats1")
            stats1_g = small.tile([128, 4 * KT], f32, tag="stats1g")
            gb_sb = small.tile([128, 2 * KT], f32, tag="gb")
            scale1 = small.tile([128, 2 * KT], f32, tag="scale1")
            shift1 = small.tile([128, 2 * KT], f32, tag="shift1")
            stats2 = small.tile([128, 4], f32, tag="stats2")
            stats2_g = small.tile([128, 4], f32, tag="stats2g")
            istd2 = small.tile([128, 2], f32, tag="istd2")
            shift2 = small.tile([128, 2], f32, tag="shift2")
            ones_k = small.tile([128, 1], f32r, tag="ones_k")
            ones_m = small.tile([1, 128], f32r, tag="ones_m")
            qloc = persist.tile([128, 2 * R], bf16, tag="qloc")
            eps_sb = small.tile([128, 1], f32, tag="eps")
            ones_k32 = small.tile([128, 1], f32, tag="ones_k32")
            ones_m32 = small.tile([1, 128], f32, tag="ones_m32")

            nc.sync.dma_start(gb_sb[:], gb.ap()[:])
            nc.vector.memset(ones_k32[:], 1.0)
            nc.vector.memset(ones_m32[:], 1.0)
            nc.vector.tensor_copy(ones_k[:], ones_k32[:])
            nc.vector.tensor_copy(ones_m[:], ones_m32[:])
            nc.vector.memset(eps_sb[:], BN_EPS)

            xs = {}
            ys = {}
            us = {}

            # ---------- emission helpers (each emits a small chunk) ----------
            def emit_ar1(zi):
                """BN1-stats AllReduce for z[zi]; bounce DMAs on GpSimd."""
                sl = slice(2 * KT * zi, 2 * KT * (zi + 1))
                nc.gpsimd.dma_start(ar1_in[zi].ap()[:], stats1[:, sl])
                nc.gpsimd.collective_compute(
                    "AllReduce", ALU.add, replica_groups=rg,
                    ins=[ar1_in[zi].ap()], outs=[ar1_out[zi].ap()])
                nc.gpsimd.dma_start(stats1_g[:, sl], ar1_out[zi].ap()[:])

            def emit_coef1(zi):
                csl = slice(KT * zi, KT * (zi + 1))
                mean1 = small.tile([128, KT], f32, tag=f"mean1_{zi}")
                var1 = small.tile([128, KT], f32, tag=f"var1_{zi}")
                tmp1 = small.tile([128, KT], f32, tag=f"tmp1_{zi}")
                nc.vector.tensor_scalar_mul(
                    mean1[:], stats1_g[:, 2 * KT * zi:2 * KT * zi + KT], 1.0 / N)
                nc.vector.tensor_scalar_mul(
                    var1[:], stats1_g[:, 2 * KT * zi + KT:2 * KT * (zi + 1)],
                    1.0 / N)
                nc.vector.tensor_mul(tmp1[:], mean1[:], mean1[:])
                nc.vector.tensor_sub(var1[:], var1[:], tmp1[:])
                nc.scalar.activation(tmp1[:], var1[:], AF.Sqrt, bias=eps_sb[:])
                nc.vector.reciprocal(var1[:], tmp1[:])  # var1 := istd
                nc.vector.tensor_mul(scale1[:, csl], var1[:], gb_sb[:, 0:KT])
                nc.vector.tensor_mul(tmp1[:], mean1[:], scale1[:, csl])
                nc.vector.tensor_sub(shift1[:, csl], gb_sb[:, KT:2 * KT], tmp1[:])

            def emit_relu(zi, j, engine="act"):
                yj = yp.tile([128, R], f32r, tag="yy")
                ys[(zi, j)] = yj
                if engine == "act":
                    nc.scalar.activation(
                        yj[:], xs[(zi, j)][:], AF.Relu,
                        scale=scale1[:, KT * zi + j:KT * zi + j + 1],
                        bias=shift1[:, KT * zi + j:KT * zi + j + 1])
                else:
                    nc.vector.tensor_scalar(
                        yj[:], xs[(zi, j)][:],
                        scale1[:, KT * zi + j:KT * zi + j + 1],
                        shift1[:, KT * zi + j:KT * zi + j + 1],
                        ALU.mult, ALU.add)
                    nc.vector.tensor_scalar_max(yj[:], yj[:], 0.0)

            def emit_gemm2(zi):
                psu = pp13.tile([128, R], f32, tag="ps13")
                for j in range(KT):
                    nc.tensor.matmul(
                        psu[:], w2sb[:, j, :], ys[(zi, j)][:],
                        start=(j == 0), stop=(j == KT - 1))
                un = up.tile([128, R], f32, tag="uu")
                us[zi] = un
                nc.vector.tensor_scalar(
                    un[:], psu[:], 0.0, 0.0, ALU.add, ALU.add,
                    accum_out=stats2[:, 2 * zi:2 * zi + 1])
                scr = scrp.tile([128, R], f32, tag="scr")
                nc.scalar.activation(
                    scr[:], psu[:], AF.Square,
                    accum_out=stats2[:, 2 * zi + 1:2 * zi + 2])

            def emit_ar2(zi):
                nc.gpsimd.dma_start(ar2_in[zi].ap()[:],
                                    stats2[:, 2 * zi:2 * (zi + 1)])
                nc.gpsimd.collective_compute(
                    "AllReduce", ALU.add, replica_groups=rg,
                    ins=[ar2_in[zi].ap()], outs=[ar2_out[zi].ap()])
                nc.gpsimd.dma_start(stats2_g[:, 2 * zi:2 * (zi + 1)],
                                    ar2_out[zi].ap()[:])

            def emit_coef2(zi):
                mean2 = small.tile([128, 1], f32, tag=f"mean2_{zi}")
                var2 = small.tile([128, 1], f32, tag=f"var2_{zi}")
                tmp2 = small.tile([128, 1], f32, tag=f"tmp2_{zi}")
                nc.vector.tensor_scalar_mul(
                    mean2[:], stats2_g[:, 2 * zi:2 * zi + 1], 1.0 / N)
                nc.vector.tensor_scalar_mul(
                    var2[:], stats2_g[:, 2 * zi + 1:2 * zi + 2], 1.0 / N)
                nc.vector.tensor_mul(tmp2[:], mean2[:], mean2[:])
                nc.vector.tensor_sub(var2[:], var2[:], tmp2[:])
                nc.scalar.activation(tmp2[:], var2[:], AF.Sqrt, bias=eps_sb[:])
                nc.vector.reciprocal(istd2[:, zi:zi + 1], tmp2[:])
                nc.vector.tensor_mul(tmp2[:], mean2[:], istd2[:, zi:zi + 1])
                nc.vector.tensor_scalar_mul(shift2[:, zi:zi + 1], tmp2[:], -1.0)

            def emit_phasec(zi):
                """P = BN2(U); qhat = P/||P||; stage + AllGather this z."""
                zsl = slice(R * zi, R * (zi + 1))
                ph = php.tile([128, R], f32, tag="ph")
                nc.vector.tensor_scalar(
                    ph[:], us[zi][:], istd2[:, zi:zi + 1],
                    shift2[:, zi:zi + 1], ALU.mult, ALU.add)
                sq = sqp.tile([128, R], f32r, tag="sq")
                nc.scalar.activation(sq[:], ph[:], AF.Square)
                n2 = ppn.tile([1, R], f32, tag="psn")
                nc.tensor.matmul(n2[:], ones_k[:], sq[:], start=True, stop=True)
                nrm = nrmp.tile([1, R], f32, tag="nrm")
                nc.scalar.activation(nrm[:], n2[:], AF.Sqrt)
                rinv = nrmp.tile([1, R], f32r, tag="rinv")
                with nc.allow_low_precision(
                        reason="f32r rounding of 1/||p|| is intentional"):
                    nc.vector.reciprocal(rinv[:], nrm[:])
                rb = ppb.tile([128, R], f32, tag="psb")
                nc.tensor.matmul(rb[:], ones_m[:], rinv[:], start=True, stop=True)
                nc.vector.tensor_mul(qloc[:, zsl], ph[:], rb[:])
                nc.gpsimd.dma_start(ag_in[zi].ap()[:], qloc[:, zsl])
                nc.gpsimd.collective_compute(
                    "AllGather", ALU.bypass, replica_groups=rg,
                    ins=[ag_in[zi].ap()], outs=[ag_out[zi].ap()])

            def emit_gemm1_j(zi, j, w1j):
                zsl = slice(R * zi, R * (zi + 1))
                xj = xp.tile([128, R], f32, tag="xx")
                xs[(zi, j)] = xj
                ps = pp13.tile([128, R], f32, tag="ps13")
                for k in range(KT):
                    nc.tensor.matmul(
                        ps[:], w1j[:, k, :], zt_k(k, zsl),
                        start=(k == 0), stop=(k == KT - 1))
                nc.vector.tensor_scalar(
                    xj[:], ps[:], 0.0, 0.0, ALU.add, ALU.add,
                    accum_out=stats1[:, 2 * KT * zi + j:2 * KT * zi + j + 1])
                scr = scrp.tile([128, R], f32, tag="scr")
                nc.scalar.activation(
                    scr[:], ps[:], AF.Square,
                    accum_out=stats1[:, 2 * KT * zi + KT + j:
                                     2 * KT * zi + KT + j + 1])

            # ---------------- main emission schedule ----------------
            with tc.tile_pool(name="w1", bufs=2) as w1p, \
                 tc.tile_pool(name="ztp", bufs=1) as ztp, \
                 tc.tile_pool(name="xx", bufs=22) as xp, \
                 tc.tile_pool(name="yy", bufs=16) as yp, \
                 tc.tile_pool(name="w2", bufs=1) as w2p, \
                 tc.tile_pool(name="scr", bufs=2) as scrp, \
                 tc.tile_pool(name="ps13", bufs=3, space="PSUM") as pp13:

                # first weight block before the big z load, so PE can start
                # as soon as zT lands
                w1_first = w1p.tile([128, KT, 128], bf16, tag="w1")
                nc.sync.dma_start(
                    w1_first[:], W1c.ap()[0].rearrange("(k p) c -> p k c", p=128))
                zta = ztp.tile([128, KT // 2, 2 * R], bf16, tag="zta")
                ztb = ztp.tile([128, KT // 2, 2 * R], bf16, tag="ztb")
                nc.sync.dma_start(
                    zta[:], zT.ap()[0:D // 2].rearrange("(k p) n -> p k n", p=128))
                nc.sync.dma_start(
                    ztb[:], zT.ap()[D // 2:D].rearrange("(k p) n -> p k n", p=128))

                def zt_k(k, zsl):
                    if k < KT // 2:
                        return zta[:, k, zsl]
                    return ztb[:, k - KT // 2, zsl]

                w2sb = w2p.tile([128, KT, 128], f32r, tag="w2")
                nc.sync.dma_start(
                    w2sb[:], W2c.ap().rearrange("(k p) c -> p k c", p=128))

                # GEMM1 for z1
                for j in range(KT):
                    if j == 0:
                        w1j = w1_first
                    else:
                        w1j = w1p.tile([128, KT, 128], bf16, tag="w1")
                        nc.sync.dma_start(
                            w1j[:],
                            W1c.ap()[j].rearrange("(k p) c -> p k c", p=128))
                    emit_gemm1_j(0, j, w1j)

                emit_ar1(0)  # hides under z2's GEMM1

                # GEMM1 for z2, with z1's dependent chain interleaved
                for j in range(KT):
                    w1j = w1p.tile([128, KT, 128], bf16, tag="w1")
                    nc.sync.dma_start(
                        w1j[:], W1c.ap()[j].rearrange("(k p) c -> p k c", p=128))
                    emit_gemm1_j(1, j, w1j)
                    if j == 1:
                        emit_coef1(0)
                    if 2 <= j <= 9:
                        emit_relu(0, 2 * (j - 2))
                        emit_relu(0, 2 * (j - 2) + 1)
                    if j == 10:
                        emit_gemm2(0)
                    if j == 11:
                        emit_ar2(0)
                    if j == 14:
                        emit_coef2(0)

                emit_ar1(1)       # z2 BN1 stats reduce (TOPSP before AG-z1)
                emit_phasec(0)    # z1 normalize + AllGather (AR2-z1 done)

                # z2 dependent chain: split relu across ACT and DVE to
                # shorten the critical path; GEMM2/AR2/phaseC-z2 emitted
                # before phase 5 so the PE and TOPSP queues don't block
                emit_coef1(1)
                for j in range(KT):
                    emit_relu(1, j, engine=("dve" if j % 3 == 2 else "act"))
                emit_gemm2(1)
                emit_ar2(1)
                emit_coef2(1)
                emit_phasec(1)

            # phase 1-3 pools released here
            if dbg:
                nc.sync.dma_start(dbg_t["qloc"].ap()[:], qloc[:])

            with tc.tile_pool(name="qf", bufs=1) as qfp, \
                 tc.tile_pool(name="stage", bufs=4) as stp, \
                 tc.tile_pool(name="ps5", bufs=3, space="PSUM") as pp5:
                # qfull columns: 4096*zb + 512*cb + i  (global column order)
                qfull = qfp.tile([128, 2 * N], bf16, tag="qf")

                def emit_qload(zb):
                    for cb in range(NCORES):
                        nc.sync.dma_start(
                            qfull[:, N * zb + R * cb:N * zb + R * (cb + 1)],
                            ag_out[zb].ap()[128 * cb:128 * (cb + 1), :])

                def emit_p5(zb, trange):
                    for t in trange:
                        lhs = qloc[:, 128 * t:128 * (t + 1)]
                        out_dram = out_top if t < 4 else out_bot
                        row0 = 128 * (t % 4)
                        for h in range(2):   # two 2048-col staging chunks
                            ob = stp.tile([128, N // 2], f32, tag="stage")
                            for q in range(4):
                                cb = 4 * h + q
                                pss = pp5.tile([128, R], f32, tag="ps5")
                                nc.tensor.matmul(
                                    pss[:], lhs,
                                    qfull[:, N * zb + R * cb:
                                          N * zb + R * (cb + 1)],
                                    start=True, stop=True)
                                osl = ob[:, R * q:R * (q + 1)]
                                if q % 2 == 0:
                                    nc.vector.tensor_scalar_mul(
                                        osl, pss[:], TEMP_SCALE)
                                else:
                                    nc.scalar.mul(osl, pss[:], TEMP_SCALE)
                            gcol = N * zb + (N // 2) * h
                            nc.sync.dma_start(
                                out_dram.ap()[row0:row0 + 128,
                                              gcol:gcol + N // 2], ob[:])

                emit_qload(0)
                if dbg:
                    nc.sync.dma_start(dbg_t["qfull"].ap()[:, 0:N], qfull[:, 0:N])
                emit_p5(0, range(0, 4))     # z1 rows x z1 cols: earliest ready
                emit_p5(0, range(4, 8))     # z2 rows x z1 cols
                emit_qload(1)
                if dbg:
                    nc.sync.dma_start(dbg_t["qfull"].ap()[:, N:2 * N], qfull[:, N:2 * N])
                emit_p5(1, range(0, 8))     # all rows x z2 cols

    nc.compile()
    return nc


def _get_nc():
    if "nc" not in _CACHE:
        _CACHE["nc"] = _build()
    return _CACHE["nc"]


def _make_in_maps(z1, z2, W1, gamma1, beta1, W2):
    W1T = np.ascontiguousarray(W1.T)                       # [k, j]
    W1c = np.ascontiguousarray(
        W1T.reshape(D, KT, 128).transpose(1, 0, 2)).astype(ml_dtypes.bfloat16)
    W2c = np.ascontiguousarray(W2.T)                       # [D, 128]
    gb = np.concatenate([gamma1.reshape(KT, 128).T,
                         beta1.reshape(KT, 128).T], axis=1).astype(np.float32)
    gb = np.ascontiguousarray(gb)                          # [128, 32]

    in_maps = []
    for c in range(NCORES):
        rs = slice(R * c, R * (c + 1))
        zTc = np.concatenate([z1[rs].T, z2[rs].T], axis=1).astype(
            ml_dtypes.bfloat16)                             # [D, 1024]
        in_maps.append({
            "zT": np.ascontiguousarray(zTc),
            "W1c": W1c, "W2c": W2c, "gb": gb,
        })
    return in_maps


def kernel(z1, z2, W1, gamma1, beta1, W2):
    z1 = np.asarray(z1, dtype=np.float32)
    z2 = np.asarray(z2, dtype=np.float32)
    W1 = np.asarray(W1, dtype=np.float32)
    W2 = np.asarray(W2, dtype=np.float32)
    gamma1 = np.asarray(gamma1, dtype=np.float32)
    beta1 = np.asarray(beta1, dtype=np.float32)

    in_maps = _make_in_maps(z1, z2, W1, gamma1, beta1, W2)
    nc = _get_nc()
    res = run_bass_kernel_spmd(nc, in_maps, core_ids=list(range(NCORES)))

    scores = np.empty((2 * N, 2 * N), dtype=np.float32)
    for c in range(NCORES):
        scores[R * c:R * (c + 1), :] = res.results[c]["out_top"]
        scores[N + R * c:N + R * (c + 1), :] = res.results[c]["out_bot"]
    idx = np.arange(N)
    scores[idx, idx] = -np.inf
    scores[N + idx, N + idx] = -np.inf
    targets = np.arange(2 * N, dtype=np.int32)
    return scores, targets
